# revision 41
# baseline (speedup 1.0000x reference)
"""Multi-head causal attention block on 8 trn2 NeuronCores.

Sharding: tensor-parallel over heads (16 heads / 8 cores = 2 heads per core).
Each core gets the full x (pre-transposed on host), its 128-wide slice of the
QKV projection columns and of the w_out rows, computes its 2 heads end to end,
and emits a partial y^T = (attn_out @ w_out_slice)^T in bf16.  Host sums the
8 partials (the "all-reduce"), transposes back, adds b_out.

Device layout (everything "transposed": head-dim on partitions, seq free):
  x^T    [128p, 8, 2048]   Q^T,K^T [128p, 2048]   V [128p(s), 16, 2, 65]
  (V natural, per head 64 hd cols + ones column so the PV matmul accumulates
  the softmax denominator for free).  scores^T [128 keys, 512 q] in PSUM; exp
  on ScalarE without max subtraction (scores ~ N(0,1)); static causal {0,1}
  masks multiplied in on VectorE for diagonal k-tiles; fully-masked tiles
  skipped.  attn_out^T = numerator^T * bcast(1/den): the denominator row is
  staged to SBUF (the custom-DVE fast reciprocal reads PSUM@partition-64
  wrong), reciprocal'd with reciprocal_approx_fast, and partition-broadcast
  on GpSimd.  Diagonal k-tiles skip fully-masked query columns (128-col
  granularity); columns below the band are never read.

Engine budget per batch: PE 56us | ACT 41us of exp + epilogues-in-holes |
DVE ~40us | Pool ~9us.  The ScalarE exp chain is gated by its own QK pairs,
so it naturally idles between exps; qk-bias epilogues and 1-in-4 projection
copies (1-in-2 on the last batch) slot into those holes, the rest of the
PSUM->SBUF copies (V tiles, most projections, den staging) go to VectorE.
Mask multiplies stay on VectorE: GpSimd's ~0.8us wake latency on the
exp->mask->PV edge collapses the PE p-state (measured 2.2x slowdown).
PV lags its exp by THREE pairs so one stray ScalarE epilogue cannot stall
the in-order PE queue.

Scheduling: the attention pair-loop (QK pair -> exp pair -> PV pair) stalls
TensorE while ScalarE exps.  Independent matmuls -- the NEXT batch's QKV
projections and the finished q-blocks' output projections -- are kept in a
FIFO of generators and dripped into those gaps, keeping TensorE dense and the
PE clock at 2.4 GHz.  The short V-projection chains interleave between the
long QK-projection chains so their ps_m slot turnarounds hide under qk
streaming.  A warmup burst of 1x64 matmuls on the ones row ramps the PE
p-state while the first x tiles are still in flight on DMA.  Q-blocks run in
DESCENDING size order for non-final batches (the 16-k-tile blocks -- most
filler slots -- run while the queue is fullest); the FINAL batch runs
ASCENDING, since it has no next-batch chains to fill with: each q-block's
out-projections interleave into the next (bigger) q-block's attention and
only the very last q-block's projs drain solo (borrowing the freed score
banks for PSUM rotation).

DMA: the Sync engine spends ~0.7us per issued descriptor, so transfers are
batched -- x arrives as one [128, 8, 512] tile per query block (per-chunk
for batch 0, which is latency-critical), and the projection outputs of a
query block are collected into one [128, 8, 512] bf16 tile and shipped with
a single DMA.  (Per-chunk tail DMAs were tried and are ~35us SLOWER: each
waits ~10us on its chunk-copy semaphore in the Sync queue, serializing the
drain at ~2.5us/chunk with the PE dropped to its cold p-state.)
Partials are emitted in bf16 (the host all-reduce absorbs the rounding).
"""

from collections import deque

import numpy as np
import ml_dtypes

B, S, D, H = 4, 2048, 1024, 16
HD = 64                      # head dim
N_CORES = 8
HPC = H // N_CORES           # heads per core = 2
HDIM = HPC * HD              # per-core qkv slice width = 128
CH = D // 128                # contraction chunks = 8
SQ = 512                     # query block
NQ = S // SQ                 # 4 query blocks
SK = 128                     # key tile
NKT = S // SK                # 16 key tiles

_CACHE = {}
FAST_RECIP = True
FILLERS = True
PV_LAG = 3


def _build(with_vbias, with_qkbias=False):
    import concourse.bass as bass
    import concourse.tile as tile
    from concourse import bacc, mybir
    from contextlib import ExitStack

    bf16 = mybir.dt.bfloat16
    f32 = mybir.dt.float32
    EXP = mybir.ActivationFunctionType.Exp

    nc = bacc.Bacc("TRN2", target_bir_lowering=False, debug=False,
                   num_devices=N_CORES)

    xt = nc.dram_tensor("xt", [B, D, S], bf16, kind="ExternalInput")
    wq = nc.dram_tensor("wq", [D, HDIM], bf16, kind="ExternalInput")
    wk = nc.dram_tensor("wk", [D, HDIM], bf16, kind="ExternalInput")
    wv = nc.dram_tensor("wv", [D, HDIM], bf16, kind="ExternalInput")
    wo = nc.dram_tensor("wo", [HDIM, D], bf16, kind="ExternalInput")
    masks = nc.dram_tensor("masks", [128, 4 * SQ], bf16, kind="ExternalInput")
    bias_qk = nc.dram_tensor("bias_qk", [128, 2], f32, kind="ExternalInput")
    bias_v = nc.dram_tensor("bias_v", [128, 2 * HD], f32, kind="ExternalInput")
    ones64 = nc.dram_tensor("ones64", [1, 64], f32, kind="ExternalInput")
    out = nc.dram_tensor("out", [B, D, S], bf16, kind="ExternalOutput")

    xt_r = xt.ap().rearrange("b (o p) s -> b p o s", p=128)
    wq_r = wq.ap().rearrange("(o p) m -> p o m", p=128)
    wk_r = wk.ap().rearrange("(o p) m -> p o m", p=128)
    wv_r = wv.ap().rearrange("(o p) m -> p o m", p=128)
    out_r = out.ap().rearrange("b (o p) s -> b p o s", p=128)

    with tile.TileContext(nc) as tc:
        with ExitStack() as ctx:
            constp = ctx.enter_context(tc.tile_pool(name="const", bufs=1))
            xtp = ctx.enter_context(tc.tile_pool(name="xt", bufs=2))
            qkp = ctx.enter_context(tc.tile_pool(name="qk", bufs=2))
            ep = ctx.enter_context(tc.tile_pool(name="e", bufs=8))
            smallp = ctx.enter_context(tc.tile_pool(name="small", bufs=3))
            yp = ctx.enter_context(tc.tile_pool(name="y", bufs=3))
            ps_s = ctx.enter_context(tc.tile_pool(name="ps_s", bufs=2, space="PSUM"))
            ps_o = ctx.enter_context(tc.tile_pool(name="ps_o", bufs=2, space="PSUM"))
            ps_m = ctx.enter_context(tc.tile_pool(name="ps_m", bufs=2, space="PSUM"))

            # ---- constants (ones first: it feeds the PE warmup loop) ----
            ones_sb = constp.tile([1, 64], f32, tag="ones", name="ones")
            nc.sync.dma_start(ones_sb[:], ones64.ap())
            wq_sb = constp.tile([128, CH, HDIM], bf16, tag="wq", name="wq")
            nc.sync.dma_start(wq_sb[:], wq_r)
            wk_sb = constp.tile([128, CH, HDIM], bf16, tag="wk", name="wk")
            nc.sync.dma_start(wk_sb[:], wk_r)
            wv_sb = constp.tile([128, CH, HDIM], bf16, tag="wv", name="wv")
            nc.sync.dma_start(wv_sb[:], wv_r)
            bqk_sb = constp.tile([128, 2], f32, tag="bqk", name="bqk")
            nc.sync.dma_start(bqk_sb[:], bias_qk.ap())
            wo_sb = constp.tile([HDIM, D], bf16, tag="wo", name="wo")
            masks_sb = constp.tile([128, 4, SQ], bf16, tag="masks", name="masks")
            bv_sb = constp.tile([128, 2, HD], f32, tag="bv", name="bv")

            # ---- PE p-state warmup: tiny matmuls on the ones row while the
            # first x tiles are still in flight on DMA ----
            warm = ps_m.tile([128, SQ], f32, tag="m", name="m")
            for _ in range(16):
                nc.tensor.matmul(warm[0:64, 0:64], ones_sb[:], ones_sb[:],
                                 start=True, stop=True)

            # ---- filler machinery ----
            # fillq: generators yielding after each matmul (PE-side steps).
            # epiq: deferred ACT/DVE epilogues (PSUM->SBUF copies); draining
            # them only at sub-block boundaries keeps the in-order ScalarE
            # queue clean for the exp chain. fill() pops one epilogue early
            # when >=2 are pending so ps_m slots keep rotating.
            fillq = deque()
            epiq = deque()

            def fill(n):
                k = 0
                if not FILLERS:
                    n = None
                while fillq and (n is None or k < n):
                    if len(epiq) >= 2:
                        epiq.popleft()()
                    try:
                        next(fillq[0])
                        k += 1
                    except StopIteration:
                        fillq.popleft()
                if n is None:
                    while epiq:
                        epiq.popleft()()

            def fill_epi():
                while epiq:
                    epiq.popleft()()

            def qk_group(t, so, which):
                sl = slice(so * SQ, (so + 1) * SQ)
                w = wq_sb if which == 0 else wk_sb
                dst = t["qt"] if which == 0 else t["kt"]
                ps = ps_m.tile([128, SQ], f32, tag="m", name="m")
                for c in range(CH):
                    nc.tensor.matmul(ps[:], w[:, c, :], t["xt"][so][:, c, :],
                                     start=(c == 0), stop=(c == CH - 1))
                    if c < CH - 1:
                        yield
                if with_qkbias:
                    epiq.append(lambda: nc.scalar.add(dst[:, sl], ps[:],
                                                      bqk_sb[:, which:which + 1]))
                else:
                    epiq.append(lambda: nc.scalar.copy(dst[:, sl], ps[:]))
                yield

            def v_group(t, st):
                so, off = divmod(st * SK, SQ)
                ps = ps_m.tile([128, SQ], f32, tag="m", name="m")
                for c in range(CH):
                    nc.tensor.matmul(ps[:, 0:HDIM],
                                     t["xt"][so][:, c, off:off + SK],
                                     wv_sb[:, c, :],
                                     start=(c == 0), stop=(c == CH - 1))
                    if c < CH - 1:
                        yield

                def epi():
                    nc.vector.tensor_copy(
                        t["vb"][:, st, :, 0:HD],
                        ps[:, 0:HDIM].rearrange("p (h d) -> p h d", d=HD))
                    if with_vbias:
                        nc.vector.tensor_add(t["vb"][:, st, :, 0:HD],
                                             t["vb"][:, st, :, 0:HD], bv_sb[:])
                epiq.append(epi)
                yield

            def proj_group(t, b, m, so):
                sl = slice(so * SQ, (so + 1) * SQ)
                # the final q-block keeps per-chunk DMAs (shorter tail);
                # all others batch the 8 chunk copies into one DMA issue to
                # keep the Sync engine's descriptor queue short.
                solo = b == B - 1 and so == NQ - 1
                # the tail clump runs after the last attention pair, so the
                # score banks are free: borrow them for odd chunks to double
                # the PSUM slots the drain rotates through
                if solo and m % 2 == 1:
                    ps = ps_s.tile([128, 2, SQ], f32, tag="s", name="s")[:, 0, :]
                else:
                    ps = ps_m.tile([128, SQ], f32, tag="m", name="m")
                nc.tensor.matmul(ps, wo_sb[:, m * 128:(m + 1) * 128],
                                 t["at"][:, sl], start=True, stop=True)

                def epi():
                    if m == 0:
                        t[("y", so)] = yp.tile([128, CH, SQ], bf16,
                                               tag="y", name="y")
                    y_sb = t[("y", so)]
                    dst = y_sb[:, m, :]
                    act_mod = 2 if solo else 4
                    if m % act_mod == 0:
                        nc.scalar.copy(dst, ps)
                    else:
                        nc.vector.tensor_copy(dst, ps)
                    # single batched DMA also for the tail q-block: the old
                    # per-chunk DMAs each stalled the Sync queue ~10us+
                    # behind their chunk-copy semaphores, serializing the
                    # drain at ~2.5us/chunk with the PE gone cold
                    if m == CH - 1:
                        nc.sync.dma_start(out_r[b, :, :, sl], y_sb[:])
                epiq.append(epi)
                yield

            tiles = {}

            def start_batch(b):
                xt_cs = [None] * NQ
                for so in range(NQ):
                    xc = xtp.tile([128, CH, SQ], bf16, tag=f"xt{so}",
                                  name=f"xt{so}")
                    sl = slice(so * SQ, (so + 1) * SQ)
                    if b == 0:
                        # batch 0 is latency-critical: per-chunk DMAs so the
                        # first qk chains start as chunks arrive
                        for c in range(CH):
                            nc.sync.dma_start(xc[:, c, :], xt_r[b, :, c, sl])
                    else:
                        nc.sync.dma_start(xc[:], xt_r[b, :, :, sl])
                    xt_cs[so] = xc
                t = {
                    "xt": xt_cs,
                    "qt": qkp.tile([128, S], bf16, tag="qt", name="qt"),
                    "kt": qkp.tile([128, S], bf16, tag="kt", name="kt"),
                    "vb": qkp.tile([128, NKT, 2, 65], bf16, tag="vb", name="vb"),
                }
                tiles[b] = t
                nc.vector.memset(t["vb"][:, :, :, HD:65], 1.0)
                # interleave the short v chains between the long qk chains so
                # the v groups' ps_m slot turnarounds hide under qk streaming
                for so in range(NQ):
                    fillq.append(qk_group(t, so, 0))
                    fillq.append(v_group(t, 4 * so + 0))
                    fillq.append(v_group(t, 4 * so + 1))
                    fillq.append(qk_group(t, so, 1))
                    fillq.append(v_group(t, 4 * so + 2))
                    fillq.append(v_group(t, 4 * so + 3))

            def attention(b):
                t = tiles[b]
                t["at"] = qkp.tile([128, S], bf16, tag="at", name="at")
                at = t["at"]
                qt, kt, vb = t["qt"], t["kt"], t["vb"]
                # non-final batches DESCENDING (16-k-tile block sees the
                # fullest filler queue); the final batch has no next-batch
                # chains, so it runs ASCENDING: each q-block's out-projs
                # interleave into the next (bigger) q-block's attention and
                # only the very last q-block's projs drain solo
                order = range(NQ) if b == B - 1 else range(NQ - 1, -1, -1)
                for qi in order:
                    qsl = slice(qi * SQ, (qi + 1) * SQ)
                    n_kt = qi * 4 + 4
                    n_pairs = n_kt // 2
                    pso = [ps_o.tile([65, SQ], f32, tag="o", name="o")
                           for _ in range(HPC)]
                    prevs = deque()

                    def emit_pv(e0, p0, c0s, h):
                        for j in range(2):
                            ki = 2 * p0 + j
                            nc.tensor.matmul(pso[h][:, c0s[j]:SQ],
                                             vb[:, ki, h, :],
                                             e0[:, j, c0s[j]:SQ],
                                             start=(ki == 0),
                                             stop=(ki == n_kt - 1))

                    for pi in range(n_pairs):
                        # diagonal k-tile at delta d: queries < d in this
                        # block are fully masked -> skip those columns.
                        # The pair's exp covers from the smaller c0; the
                        # skipped-but-exp'd region holds stale bounded
                        # scores and is zeroed by the mask multiply.
                        c0s = []
                        for j in range(2):
                            didx = 2 * pi + j - qi * 4
                            c0s.append(didx * SK if didx > 0 else 0)
                        ce = min(c0s)
                        psp = [ps_s.tile([128, 2, SQ], f32, tag="s", name="s")
                               for _ in range(HPC)]
                        # j-major, head-minor emission: consecutive QK
                        # matmuls alternate PE row-groups (head0 at SBUF
                        # partitions 0-63 -> tile rows 0-63, head1 at
                        # 64-127), so each h0/h1 pair can run concurrently
                        # in disjoint halves of the systolic array
                        for j in range(2):
                            ki = 2 * pi + j
                            for h in range(HPC):
                                hsl = slice(h * HD, (h + 1) * HD)
                                nc.tensor.matmul(
                                    psp[h][:, j, c0s[j]:SQ],
                                    kt[hsl, ki * SK:(ki + 1) * SK],
                                    qt[hsl, qi * SQ + c0s[j]:(qi + 1) * SQ],
                                    start=True, stop=True)
                        fill(1)
                        for h in range(HPC):
                            epair = ep.tile([128, 2, SQ], bf16, tag="e",
                                            name="e")
                            nc.scalar.activation(epair[:, :, ce:SQ],
                                                 psp[h][:, :, ce:SQ], EXP)
                            for j in range(2):
                                didx = 2 * pi + j - qi * 4
                                if didx >= 0:
                                    dd = didx * SK
                                    nc.vector.tensor_mul(
                                        epair[:, j, dd:dd + SK],
                                        epair[:, j, dd:dd + SK],
                                        masks_sb[:, didx, dd:dd + SK])
                            fill(1)
                            if len(prevs) >= PV_LAG:
                                emit_pv(*prevs.popleft())
                                fill(1)
                            prevs.append((epair, pi, c0s, h))
                    while prevs:
                        emit_pv(*prevs.popleft())

                    # normalize: at[hd, q] = num[hd, q] * bcast(1/den[q])
                    for h in range(HPC):
                        hsl = slice(h * HD, (h + 1) * HD)
                        recip = smallp.tile([1, SQ], f32, tag="recip",
                                            name="recip")
                        if FAST_RECIP:
                            den = smallp.tile([1, SQ], f32, tag="den",
                                              name="den")
                            nc.vector.tensor_copy(den[:], pso[h][64:65, :])
                            nc.vector.reciprocal_approx_fast(out=recip[:],
                                                             in_=den[:])
                        else:
                            nc.vector.reciprocal(recip[:], pso[h][64:65, :])
                        bc = smallp.tile([64, SQ], f32, tag="bc", name="bc")
                        nc.gpsimd.partition_broadcast(bc[:], recip[:],
                                                      channels=64)
                        nc.vector.tensor_mul(at[hsl, qsl], pso[h][0:64, :],
                                             bc[:])
                        fill_epi()
                        fill(4)
                    for m in range(CH):
                        fillq.append(proj_group(t, b, m, qi))
                    fill_epi()
                fill(None)

            start_batch(0)
            nc.sync.dma_start(masks_sb[:], masks.ap().rearrange("p (d q) -> p d q", q=SQ))
            nc.sync.dma_start(wo_sb[:], wo.ap())
            nc.sync.dma_start(bv_sb[:], bias_v.ap().rearrange("p (h d) -> p h d", d=HD))
            fill(None)
            for b in range(B):
                if b + 1 < B:
                    start_batch(b + 1)
                attention(b)

    nc.compile()
    return nc


def _get_nc(with_vbias=False, with_qkbias=False):
    key = ("nc", with_vbias, with_qkbias, FAST_RECIP, FILLERS)
    if key not in _CACHE:
        _CACHE[key] = _build(with_vbias, with_qkbias)
    return _CACHE[key]


def _prep_in_maps(x, w_in, b_in, w_out):
    bf16 = ml_dtypes.bfloat16
    scale = 1.0 / np.sqrt(HD)
    xt_host = np.ascontiguousarray(x.transpose(0, 2, 1)).astype(bf16)

    # mask[p, d*SQ + q] = 1 if key (d*128 + p) <= query q within the block
    p_idx = np.arange(128)[:, None]
    q_idx = np.arange(SQ)[None, :]
    mask_host = np.concatenate(
        [(p_idx + d * SK <= q_idx) for d in range(4)], axis=1).astype(bf16)
    ones_host = np.ones((1, 64), np.float32)

    in_maps = []
    for c in range(N_CORES):
        cs = c * HDIM
        wq_c = np.ascontiguousarray(w_in[:, cs:cs + HDIM] * scale).astype(bf16)
        wk_c = np.ascontiguousarray(w_in[:, D + cs:D + cs + HDIM]).astype(bf16)
        wv_c = np.ascontiguousarray(w_in[:, 2 * D + cs:2 * D + cs + HDIM]).astype(bf16)
        wo_c = np.ascontiguousarray(w_out[cs:cs + HDIM, :]).astype(bf16)
        bqk_c = np.ascontiguousarray(
            np.stack([b_in[cs:cs + HDIM] * scale,
                      b_in[D + cs:D + cs + HDIM]], axis=1).astype(np.float32))
        bv_c = np.ascontiguousarray(
            np.broadcast_to(b_in[2 * D + cs:2 * D + cs + HDIM],
                            (128, HDIM)).astype(np.float32))
        in_maps.append({
            "xt": xt_host, "wq": wq_c, "wk": wk_c, "wv": wv_c, "wo": wo_c,
            "masks": mask_host, "bias_qk": bqk_c, "bias_v": bv_c,
            "ones64": ones_host,
        })
    return in_maps


def kernel(x, w_in, b_in, w_out, b_out):
    from concourse.bass_utils import run_bass_kernel_spmd

    x = np.asarray(x, dtype=np.float32)
    w_in = np.asarray(w_in, dtype=np.float32)
    b_in = np.asarray(b_in, dtype=np.float32)
    w_out = np.asarray(w_out, dtype=np.float32)
    b_out = np.asarray(b_out, dtype=np.float32)

    with_vbias = bool(np.any(b_in[2 * D:]))
    with_qkbias = bool(np.any(b_in[:2 * D]))
    nc = _get_nc(with_vbias, with_qkbias)
    in_maps = _prep_in_maps(x, w_in, b_in, w_out)
    _CACHE["in_maps"] = in_maps

    res = run_bass_kernel_spmd(nc, in_maps, core_ids=list(range(N_CORES)))
    y_t = res.results[0]["out"].astype(np.float32)
    for c in range(1, N_CORES):
        y_t += res.results[c]["out"].astype(np.float32)
    y = y_t.transpose(0, 2, 1).astype(np.float32) + b_out
    return y



# revision 43
# speedup vs baseline: 1.0334x; 1.0334x over previous
"""Multi-head causal attention block on 8 trn2 NeuronCores.

Sharding: tensor-parallel over heads (16 heads / 8 cores = 2 heads per core).
Each core gets the full x (pre-transposed on host), its 128-wide slice of the
QKV projection columns and of the w_out rows, computes its 2 heads end to end,
and emits a partial y^T = (attn_out @ w_out_slice)^T in bf16.  Host sums the
8 partials (the "all-reduce"), transposes back, adds b_out.

Device layout (everything "transposed": head-dim on partitions, seq free):
  x^T    [128p, 8, 2048]   Q^T,K^T [128p, 2048]   V [128p(s), 16, 2, 65]
  (V natural, per head 64 hd cols + ones column so the PV matmul accumulates
  the softmax denominator for free).  scores^T [128 keys, 512 q] in PSUM; exp
  on ScalarE without max subtraction (scores ~ N(0,1)); static causal {0,1}
  masks multiplied in on VectorE for diagonal k-tiles; fully-masked tiles
  skipped.  attn_out^T = numerator^T * bcast(1/den): the denominator row is
  staged to SBUF (the custom-DVE fast reciprocal reads PSUM@partition-64
  wrong), reciprocal'd with reciprocal_approx_fast, and partition-broadcast
  on GpSimd.  Diagonal k-tiles skip fully-masked query columns (128-col
  granularity); columns below the band are never read.

Engine budget per batch: PE 56us | ACT 41us of exp + epilogues-in-holes |
DVE ~40us | Pool ~9us.  The ScalarE exp chain is gated by its own QK pairs,
so it naturally idles between exps; qk-bias epilogues and 1-in-4 projection
copies (1-in-2 on the last batch) slot into those holes, the rest of the
PSUM->SBUF copies (V tiles, most projections, den staging) go to VectorE.
Mask multiplies stay on VectorE: GpSimd's ~0.8us wake latency on the
exp->mask->PV edge collapses the PE p-state (measured 2.2x slowdown).
PV lags its exp by THREE pairs so one stray ScalarE epilogue cannot stall
the in-order PE queue.

Scheduling: the attention pair-loop (QK pair -> exp pair -> PV pair) stalls
TensorE while ScalarE exps.  Independent matmuls -- the NEXT batch's QKV
projections and the finished q-blocks' output projections -- are kept in a
FIFO of generators and dripped into those gaps, keeping TensorE dense and the
PE clock at 2.4 GHz.  The short V-projection chains interleave between the
long QK-projection chains so their ps_m slot turnarounds hide under qk
streaming.  A warmup burst of 1x64 matmuls on the ones row ramps the PE
p-state while the first x tiles are still in flight on DMA.  Q-blocks run in
DESCENDING size order for non-final batches (the 16-k-tile blocks -- most
filler slots -- run while the queue is fullest); the FINAL batch runs
ASCENDING, since it has no next-batch chains to fill with: each q-block's
out-projections interleave into the next (bigger) q-block's attention and
only the very last q-block's projs drain solo (borrowing the freed score
banks for PSUM rotation).

DMA: the Sync engine spends ~0.7us per issued descriptor, so transfers are
batched -- x arrives as one [128, 8, 512] tile per query block (per-chunk
for batch 0, which is latency-critical), and the projection outputs of a
query block are collected into one [128, 8, 512] bf16 tile and shipped with
a single DMA.  (Per-chunk tail DMAs were tried and are ~35us SLOWER: each
waits ~10us on its chunk-copy semaphore in the Sync queue, serializing the
drain at ~2.5us/chunk with the PE dropped to its cold p-state.)
Partials are emitted in bf16 (the host all-reduce absorbs the rounding).
"""

from collections import deque

import numpy as np
import ml_dtypes

B, S, D, H = 4, 2048, 1024, 16
HD = 64                      # head dim
N_CORES = 8
HPC = H // N_CORES           # heads per core = 2
HDIM = HPC * HD              # per-core qkv slice width = 128
CH = D // 128                # contraction chunks = 8
SQ = 512                     # query block
NQ = S // SQ                 # 4 query blocks
SK = 128                     # key tile
NKT = S // SK                # 16 key tiles

_CACHE = {}
FAST_RECIP = True
FILLERS = True
PV_LAG = 3


def _build(with_vbias, with_qkbias=False):
    import concourse.bass as bass
    import concourse.tile as tile
    from concourse import bacc, mybir
    from contextlib import ExitStack

    bf16 = mybir.dt.bfloat16
    f32 = mybir.dt.float32
    EXP = mybir.ActivationFunctionType.Exp

    nc = bacc.Bacc("TRN2", target_bir_lowering=False, debug=False,
                   num_devices=N_CORES)

    xt = nc.dram_tensor("xt", [B, D, S], bf16, kind="ExternalInput")
    wq = nc.dram_tensor("wq", [D, HDIM], bf16, kind="ExternalInput")
    wk = nc.dram_tensor("wk", [D, HDIM], bf16, kind="ExternalInput")
    wv = nc.dram_tensor("wv", [D, HDIM], bf16, kind="ExternalInput")
    wo = nc.dram_tensor("wo", [HDIM, D], bf16, kind="ExternalInput")
    masks = nc.dram_tensor("masks", [128, 4 * SQ], bf16, kind="ExternalInput")
    bias_qk = nc.dram_tensor("bias_qk", [128, 2], f32, kind="ExternalInput")
    bias_v = nc.dram_tensor("bias_v", [128, 2 * HD], f32, kind="ExternalInput")
    ones64 = nc.dram_tensor("ones64", [1, 64], f32, kind="ExternalInput")
    out = nc.dram_tensor("out", [B, D, S], bf16, kind="ExternalOutput")

    xt_r = xt.ap().rearrange("b (o p) s -> b p o s", p=128)
    wq_r = wq.ap().rearrange("(o p) m -> p o m", p=128)
    wk_r = wk.ap().rearrange("(o p) m -> p o m", p=128)
    wv_r = wv.ap().rearrange("(o p) m -> p o m", p=128)
    out_r = out.ap().rearrange("b (o p) s -> b p o s", p=128)

    with tile.TileContext(nc) as tc:
        with ExitStack() as ctx:
            constp = ctx.enter_context(tc.tile_pool(name="const", bufs=1))
            xtp = ctx.enter_context(tc.tile_pool(name="xt", bufs=2))
            qkp = ctx.enter_context(tc.tile_pool(name="qk", bufs=2))
            ep = ctx.enter_context(tc.tile_pool(name="e", bufs=8))
            smallp = ctx.enter_context(tc.tile_pool(name="small", bufs=3))
            yp = ctx.enter_context(tc.tile_pool(name="y", bufs=3))
            ps_s = ctx.enter_context(tc.tile_pool(name="ps_s", bufs=2, space="PSUM"))
            ps_o = ctx.enter_context(tc.tile_pool(name="ps_o", bufs=2, space="PSUM"))
            ps_m = ctx.enter_context(tc.tile_pool(name="ps_m", bufs=2, space="PSUM"))

            # ---- constants (ones first: it feeds the PE warmup loop) ----
            ones_sb = constp.tile([1, 64], f32, tag="ones", name="ones")
            nc.sync.dma_start(ones_sb[:], ones64.ap())
            wq_sb = constp.tile([128, CH, HDIM], bf16, tag="wq", name="wq")
            nc.sync.dma_start(wq_sb[:], wq_r)
            wk_sb = constp.tile([128, CH, HDIM], bf16, tag="wk", name="wk")
            nc.sync.dma_start(wk_sb[:], wk_r)
            wv_sb = constp.tile([128, CH, HDIM], bf16, tag="wv", name="wv")
            nc.sync.dma_start(wv_sb[:], wv_r)
            bqk_sb = constp.tile([128, 2], f32, tag="bqk", name="bqk")
            nc.sync.dma_start(bqk_sb[:], bias_qk.ap())
            wo_sb = constp.tile([HDIM, D], bf16, tag="wo", name="wo")
            masks_sb = constp.tile([128, 4, SQ], bf16, tag="masks", name="masks")
            bv_sb = constp.tile([128, 2, HD], f32, tag="bv", name="bv")

            # ---- PE p-state warmup: tiny matmuls on the ones row while the
            # first x tiles are still in flight on DMA.  48, not 16: the
            # trace shows 16 end ~3us BEFORE the first x chunk lands, HAM
            # re-throttles in that idle window, and the whole ~27us chain
            # startup then runs at the cold 1.2 GHz p-state (~432ns/MM).
            # Once warm (~11 MMs) each extra 1x64 matmul costs ~100ns, so
            # the burst ends right as the first chain matmuls are ready. ----
            warm = ps_m.tile([128, SQ], f32, tag="m", name="m")
            for _ in range(48):
                nc.tensor.matmul(warm[0:64, 0:64], ones_sb[:], ones_sb[:],
                                 start=True, stop=True)

            # ---- filler machinery ----
            # fillq: generators yielding after each matmul (PE-side steps).
            # epiq: deferred ACT/DVE epilogues (PSUM->SBUF copies); draining
            # them only at sub-block boundaries keeps the in-order ScalarE
            # queue clean for the exp chain. fill() pops one epilogue early
            # when >=2 are pending so ps_m slots keep rotating.
            fillq = deque()
            epiq = deque()

            def fill(n):
                k = 0
                if not FILLERS:
                    n = None
                while fillq and (n is None or k < n):
                    if len(epiq) >= 2:
                        epiq.popleft()()
                    try:
                        next(fillq[0])
                        k += 1
                    except StopIteration:
                        fillq.popleft()
                if n is None:
                    while epiq:
                        epiq.popleft()()

            def fill_epi():
                while epiq:
                    epiq.popleft()()

            def qk_group(t, so, which):
                sl = slice(so * SQ, (so + 1) * SQ)
                w = wq_sb if which == 0 else wk_sb
                dst = t["qt"] if which == 0 else t["kt"]
                ps = ps_m.tile([128, SQ], f32, tag="m", name="m")
                for c in range(CH):
                    nc.tensor.matmul(ps[:], w[:, c, :], t["xt"][so][:, c, :],
                                     start=(c == 0), stop=(c == CH - 1))
                    if c < CH - 1:
                        yield
                if with_qkbias:
                    epiq.append(lambda: nc.scalar.add(dst[:, sl], ps[:],
                                                      bqk_sb[:, which:which + 1]))
                else:
                    epiq.append(lambda: nc.scalar.copy(dst[:, sl], ps[:]))
                yield

            def v_group(t, st):
                so, off = divmod(st * SK, SQ)
                ps = ps_m.tile([128, SQ], f32, tag="m", name="m")
                for c in range(CH):
                    nc.tensor.matmul(ps[:, 0:HDIM],
                                     t["xt"][so][:, c, off:off + SK],
                                     wv_sb[:, c, :],
                                     start=(c == 0), stop=(c == CH - 1))
                    if c < CH - 1:
                        yield

                def epi():
                    nc.vector.tensor_copy(
                        t["vb"][:, st, :, 0:HD],
                        ps[:, 0:HDIM].rearrange("p (h d) -> p h d", d=HD))
                    if with_vbias:
                        nc.vector.tensor_add(t["vb"][:, st, :, 0:HD],
                                             t["vb"][:, st, :, 0:HD], bv_sb[:])
                epiq.append(epi)
                yield

            def proj_group(t, b, m, so):
                sl = slice(so * SQ, (so + 1) * SQ)
                # the final q-block keeps per-chunk DMAs (shorter tail);
                # all others batch the 8 chunk copies into one DMA issue to
                # keep the Sync engine's descriptor queue short.
                solo = b == B - 1 and so == NQ - 1
                # the tail clump runs after the last attention pair, so the
                # score banks are free: borrow them for odd chunks to double
                # the PSUM slots the drain rotates through
                if solo and m % 2 == 1:
                    ps = ps_s.tile([128, 2, SQ], f32, tag="s", name="s")[:, 0, :]
                else:
                    ps = ps_m.tile([128, SQ], f32, tag="m", name="m")
                nc.tensor.matmul(ps, wo_sb[:, m * 128:(m + 1) * 128],
                                 t["at"][:, sl], start=True, stop=True)

                def epi():
                    if m == 0:
                        t[("y", so)] = yp.tile([128, CH, SQ], bf16,
                                               tag="y", name="y")
                    y_sb = t[("y", so)]
                    dst = y_sb[:, m, :]
                    act_mod = 2 if solo else 4
                    if m % act_mod == 0:
                        nc.scalar.copy(dst, ps)
                    else:
                        nc.vector.tensor_copy(dst, ps)
                    # single batched DMA also for the tail q-block: the old
                    # per-chunk DMAs each stalled the Sync queue ~10us+
                    # behind their chunk-copy semaphores, serializing the
                    # drain at ~2.5us/chunk with the PE gone cold
                    if m == CH - 1:
                        nc.sync.dma_start(out_r[b, :, :, sl], y_sb[:])
                epiq.append(epi)
                yield

            tiles = {}

            def start_batch(b):
                xt_cs = [None] * NQ
                for so in range(NQ):
                    xc = xtp.tile([128, CH, SQ], bf16, tag=f"xt{so}",
                                  name=f"xt{so}")
                    sl = slice(so * SQ, (so + 1) * SQ)
                    if b == 0:
                        # batch 0 is latency-critical: per-chunk DMAs so the
                        # first qk chains start as chunks arrive
                        for c in range(CH):
                            nc.sync.dma_start(xc[:, c, :], xt_r[b, :, c, sl])
                    else:
                        nc.sync.dma_start(xc[:], xt_r[b, :, :, sl])
                    xt_cs[so] = xc
                t = {
                    "xt": xt_cs,
                    "qt": qkp.tile([128, S], bf16, tag="qt", name="qt"),
                    "kt": qkp.tile([128, S], bf16, tag="kt", name="kt"),
                    "vb": qkp.tile([128, NKT, 2, 65], bf16, tag="vb", name="vb"),
                }
                tiles[b] = t
                nc.vector.memset(t["vb"][:, :, :, HD:65], 1.0)
                # interleave the short v chains between the long qk chains so
                # the v groups' ps_m slot turnarounds hide under qk streaming
                for so in range(NQ):
                    fillq.append(qk_group(t, so, 0))
                    fillq.append(v_group(t, 4 * so + 0))
                    fillq.append(v_group(t, 4 * so + 1))
                    fillq.append(qk_group(t, so, 1))
                    fillq.append(v_group(t, 4 * so + 2))
                    fillq.append(v_group(t, 4 * so + 3))

            def attention(b):
                t = tiles[b]
                t["at"] = qkp.tile([128, S], bf16, tag="at", name="at")
                at = t["at"]
                qt, kt, vb = t["qt"], t["kt"], t["vb"]
                # non-final batches DESCENDING (16-k-tile block sees the
                # fullest filler queue); the final batch has no next-batch
                # chains, so it runs ASCENDING: each q-block's out-projs
                # interleave into the next (bigger) q-block's attention and
                # only the very last q-block's projs drain solo
                order = range(NQ) if b == B - 1 else range(NQ - 1, -1, -1)
                for qi in order:
                    qsl = slice(qi * SQ, (qi + 1) * SQ)
                    for h in range(HPC):
                        hsl = slice(h * HD, (h + 1) * HD)
                        n_kt = qi * 4 + 4
                        n_pairs = n_kt // 2
                        pso = ps_o.tile([65, SQ], f32, tag="o", name="o")
                        prevs = deque()

                        def emit_pv(e0, p0, c0s):
                            for j in range(2):
                                ki = 2 * p0 + j
                                nc.tensor.matmul(pso[:, c0s[j]:SQ], vb[:, ki, h, :],
                                                 e0[:, j, c0s[j]:SQ],
                                                 start=(ki == 0),
                                                 stop=(ki == n_kt - 1))

                        for pi in range(n_pairs):
                            # diagonal k-tile at delta d: queries < d in this
                            # block are fully masked -> skip those columns.
                            # The pair's exp covers from the smaller c0; the
                            # skipped-but-exp'd region holds stale bounded
                            # scores and is zeroed by the mask multiply.
                            c0s = []
                            for j in range(2):
                                didx = 2 * pi + j - qi * 4
                                c0s.append(didx * SK if didx > 0 else 0)
                            ce = min(c0s)
                            psp = ps_s.tile([128, 2, SQ], f32, tag="s", name="s")
                            for j in range(2):
                                ki = 2 * pi + j
                                nc.tensor.matmul(psp[:, j, c0s[j]:SQ],
                                                 kt[hsl, ki * SK:(ki + 1) * SK],
                                                 qt[hsl, qi * SQ + c0s[j]:(qi + 1) * SQ],
                                                 start=True, stop=True)
                            fill(1)
                            epair = ep.tile([128, 2, SQ], bf16, tag="e", name="e")
                            nc.scalar.activation(epair[:, :, ce:SQ], psp[:, :, ce:SQ], EXP)
                            for j in range(2):
                                didx = 2 * pi + j - qi * 4
                                if didx >= 0:
                                    dd = didx * SK
                                    nc.vector.tensor_mul(
                                        epair[:, j, dd:dd + SK],
                                        epair[:, j, dd:dd + SK],
                                        masks_sb[:, didx, dd:dd + SK])
                            fill(1)
                            if len(prevs) >= PV_LAG:
                                emit_pv(*prevs.popleft())
                                fill(1)
                            prevs.append((epair, pi, c0s))
                        while prevs:
                            emit_pv(*prevs.popleft())

                        # normalize: at[hd, q] = num[hd, q] * bcast(1/den[q])
                        recip = smallp.tile([1, SQ], f32, tag="recip", name="recip")
                        if FAST_RECIP:
                            den = smallp.tile([1, SQ], f32, tag="den", name="den")
                            nc.vector.tensor_copy(den[:], pso[64:65, :])
                            nc.vector.reciprocal_approx_fast(out=recip[:],
                                                             in_=den[:])
                        else:
                            nc.vector.reciprocal(recip[:], pso[64:65, :])
                        bc = smallp.tile([64, SQ], f32, tag="bc", name="bc")
                        nc.gpsimd.partition_broadcast(bc[:], recip[:], channels=64)
                        nc.vector.tensor_mul(at[hsl, qsl], pso[0:64, :], bc[:])
                        fill_epi()
                        fill(4)
                    for m in range(CH):
                        fillq.append(proj_group(t, b, m, qi))
                    fill_epi()
                fill(None)

            start_batch(0)
            nc.sync.dma_start(masks_sb[:], masks.ap().rearrange("p (d q) -> p d q", q=SQ))
            nc.sync.dma_start(wo_sb[:], wo.ap())
            nc.sync.dma_start(bv_sb[:], bias_v.ap().rearrange("p (h d) -> p h d", d=HD))
            fill(None)
            for b in range(B):
                if b + 1 < B:
                    start_batch(b + 1)
                attention(b)

    nc.compile()
    return nc


def _get_nc(with_vbias=False, with_qkbias=False):
    key = ("nc", with_vbias, with_qkbias, FAST_RECIP, FILLERS)
    if key not in _CACHE:
        _CACHE[key] = _build(with_vbias, with_qkbias)
    return _CACHE[key]


def _prep_in_maps(x, w_in, b_in, w_out):
    bf16 = ml_dtypes.bfloat16
    scale = 1.0 / np.sqrt(HD)
    xt_host = np.ascontiguousarray(x.transpose(0, 2, 1)).astype(bf16)

    # mask[p, d*SQ + q] = 1 if key (d*128 + p) <= query q within the block
    p_idx = np.arange(128)[:, None]
    q_idx = np.arange(SQ)[None, :]
    mask_host = np.concatenate(
        [(p_idx + d * SK <= q_idx) for d in range(4)], axis=1).astype(bf16)
    ones_host = np.ones((1, 64), np.float32)

    in_maps = []
    for c in range(N_CORES):
        cs = c * HDIM
        wq_c = np.ascontiguousarray(w_in[:, cs:cs + HDIM] * scale).astype(bf16)
        wk_c = np.ascontiguousarray(w_in[:, D + cs:D + cs + HDIM]).astype(bf16)
        wv_c = np.ascontiguousarray(w_in[:, 2 * D + cs:2 * D + cs + HDIM]).astype(bf16)
        wo_c = np.ascontiguousarray(w_out[cs:cs + HDIM, :]).astype(bf16)
        bqk_c = np.ascontiguousarray(
            np.stack([b_in[cs:cs + HDIM] * scale,
                      b_in[D + cs:D + cs + HDIM]], axis=1).astype(np.float32))
        bv_c = np.ascontiguousarray(
            np.broadcast_to(b_in[2 * D + cs:2 * D + cs + HDIM],
                            (128, HDIM)).astype(np.float32))
        in_maps.append({
            "xt": xt_host, "wq": wq_c, "wk": wk_c, "wv": wv_c, "wo": wo_c,
            "masks": mask_host, "bias_qk": bqk_c, "bias_v": bv_c,
            "ones64": ones_host,
        })
    return in_maps


def kernel(x, w_in, b_in, w_out, b_out):
    from concourse.bass_utils import run_bass_kernel_spmd

    x = np.asarray(x, dtype=np.float32)
    w_in = np.asarray(w_in, dtype=np.float32)
    b_in = np.asarray(b_in, dtype=np.float32)
    w_out = np.asarray(w_out, dtype=np.float32)
    b_out = np.asarray(b_out, dtype=np.float32)

    with_vbias = bool(np.any(b_in[2 * D:]))
    with_qkbias = bool(np.any(b_in[:2 * D]))
    nc = _get_nc(with_vbias, with_qkbias)
    in_maps = _prep_in_maps(x, w_in, b_in, w_out)
    _CACHE["in_maps"] = in_maps

    res = run_bass_kernel_spmd(nc, in_maps, core_ids=list(range(N_CORES)))
    y_t = res.results[0]["out"].astype(np.float32)
    for c in range(1, N_CORES):
        y_t += res.results[c]["out"].astype(np.float32)
    y = y_t.transpose(0, 2, 1).astype(np.float32) + b_out
    return y



# revision 45
# speedup vs baseline: 1.0397x; 1.0060x over previous
"""Multi-head causal attention block on 8 trn2 NeuronCores.

Sharding: tensor-parallel over heads (16 heads / 8 cores = 2 heads per core).
Each core gets the full x (pre-transposed on host), its 128-wide slice of the
QKV projection columns and of the w_out rows, computes its 2 heads end to end,
and emits a partial y^T = (attn_out @ w_out_slice)^T in bf16.  Host sums the
8 partials (the "all-reduce"), transposes back, adds b_out.

Device layout (everything "transposed": head-dim on partitions, seq free):
  x^T    [128p, 8, 2048]   Q^T,K^T [128p, 2048]   V [128p(s), 16, 2, 65]
  (V natural, per head 64 hd cols + ones column so the PV matmul accumulates
  the softmax denominator for free).  scores^T [128 keys, 512 q] in PSUM; exp
  on ScalarE without max subtraction (scores ~ N(0,1)); static causal {0,1}
  masks multiplied in on VectorE for diagonal k-tiles; fully-masked tiles
  skipped.  attn_out^T = numerator^T * bcast(1/den): the denominator row is
  staged to SBUF (the custom-DVE fast reciprocal reads PSUM@partition-64
  wrong), reciprocal'd with reciprocal_approx_fast, and partition-broadcast
  on GpSimd.  Diagonal k-tiles skip fully-masked query columns (128-col
  granularity); columns below the band are never read.

Engine budget per batch: PE 56us | ACT 41us of exp + epilogues-in-holes |
DVE ~40us | Pool ~9us.  The ScalarE exp chain is gated by its own QK pairs,
so it naturally idles between exps; qk-bias epilogues and 1-in-4 projection
copies (1-in-2 on the last batch) slot into those holes, the rest of the
PSUM->SBUF copies (V tiles, most projections, den staging) go to VectorE.
Mask multiplies stay on VectorE: GpSimd's ~0.8us wake latency on the
exp->mask->PV edge collapses the PE p-state (measured 2.2x slowdown).
PV lags its exp by THREE pairs so one stray ScalarE epilogue cannot stall
the in-order PE queue.

Scheduling: the attention pair-loop (QK pair -> exp pair -> PV pair) stalls
TensorE while ScalarE exps.  Independent matmuls -- the NEXT batch's QKV
projections and the finished q-blocks' output projections -- are kept in a
FIFO of generators and dripped into those gaps, keeping TensorE dense and the
PE clock at 2.4 GHz.  The short V-projection chains interleave between the
long QK-projection chains so their ps_m slot turnarounds hide under qk
streaming.  A warmup burst of 1x64 matmuls on the ones row ramps the PE
p-state while the first x tiles are still in flight on DMA.  Q-blocks run in
DESCENDING size order for non-final batches (the 16-k-tile blocks -- most
filler slots -- run while the queue is fullest); the FINAL batch runs
ASCENDING, since it has no next-batch chains to fill with: each q-block's
out-projections interleave into the next (bigger) q-block's attention and
only the very last q-block's projs drain solo (borrowing the freed score
banks for PSUM rotation).

DMA: the Sync engine spends ~0.7us per issued descriptor, so transfers are
batched -- x arrives as one [128, 8, 512] tile per query block (per-chunk
for batch 0, which is latency-critical), and the projection outputs of a
query block are collected into one [128, 8, 512] bf16 tile and shipped with
a single DMA.  (Per-chunk tail DMAs were tried and are ~35us SLOWER: each
waits ~10us on its chunk-copy semaphore in the Sync queue, serializing the
drain at ~2.5us/chunk with the PE dropped to its cold p-state.)
Partials are emitted in bf16 (the host all-reduce absorbs the rounding).
"""

from collections import deque

import numpy as np
import ml_dtypes

B, S, D, H = 4, 2048, 1024, 16
HD = 64                      # head dim
N_CORES = 8
HPC = H // N_CORES           # heads per core = 2
HDIM = HPC * HD              # per-core qkv slice width = 128
CH = D // 128                # contraction chunks = 8
SQ = 512                     # query block
NQ = S // SQ                 # 4 query blocks
SK = 128                     # key tile
NKT = S // SK                # 16 key tiles

_CACHE = {}
FAST_RECIP = True
FILLERS = True
PV_LAG = 3


def _build(with_vbias, with_qkbias=False):
    import concourse.bass as bass
    import concourse.tile as tile
    from concourse import bacc, mybir
    from contextlib import ExitStack

    bf16 = mybir.dt.bfloat16
    f32 = mybir.dt.float32
    EXP = mybir.ActivationFunctionType.Exp

    nc = bacc.Bacc("TRN2", target_bir_lowering=False, debug=False,
                   num_devices=N_CORES)

    xt = nc.dram_tensor("xt", [B, D, S], bf16, kind="ExternalInput")
    wq = nc.dram_tensor("wq", [D, HDIM], bf16, kind="ExternalInput")
    wk = nc.dram_tensor("wk", [D, HDIM], bf16, kind="ExternalInput")
    wv = nc.dram_tensor("wv", [D, HDIM], bf16, kind="ExternalInput")
    wo = nc.dram_tensor("wo", [HDIM, D], bf16, kind="ExternalInput")
    masks = nc.dram_tensor("masks", [128, 4 * SQ], bf16, kind="ExternalInput")
    bias_qk = nc.dram_tensor("bias_qk", [128, 2], f32, kind="ExternalInput")
    bias_v = nc.dram_tensor("bias_v", [128, 2 * HD], f32, kind="ExternalInput")
    ones64 = nc.dram_tensor("ones64", [1, 64], f32, kind="ExternalInput")
    out = nc.dram_tensor("out", [B, D, S], bf16, kind="ExternalOutput")

    xt_r = xt.ap().rearrange("b (o p) s -> b p o s", p=128)
    wq_r = wq.ap().rearrange("(o p) m -> p o m", p=128)
    wk_r = wk.ap().rearrange("(o p) m -> p o m", p=128)
    wv_r = wv.ap().rearrange("(o p) m -> p o m", p=128)
    out_r = out.ap().rearrange("b (o p) s -> b p o s", p=128)

    with tile.TileContext(nc) as tc:
        with ExitStack() as ctx:
            constp = ctx.enter_context(tc.tile_pool(name="const", bufs=1))
            xtp = ctx.enter_context(tc.tile_pool(name="xt", bufs=2))
            qkp = ctx.enter_context(tc.tile_pool(name="qk", bufs=2))
            ep = ctx.enter_context(tc.tile_pool(name="e", bufs=8))
            smallp = ctx.enter_context(tc.tile_pool(name="small", bufs=3))
            yp = ctx.enter_context(tc.tile_pool(name="y", bufs=3))
            ps_s = ctx.enter_context(tc.tile_pool(name="ps_s", bufs=2, space="PSUM"))
            ps_o = ctx.enter_context(tc.tile_pool(name="ps_o", bufs=2, space="PSUM"))
            ps_m = ctx.enter_context(tc.tile_pool(name="ps_m", bufs=2, space="PSUM"))

            # ---- constants (ones first: it feeds the PE warmup loop) ----
            ones_sb = constp.tile([1, 64], f32, tag="ones", name="ones")
            nc.sync.dma_start(ones_sb[:], ones64.ap())
            wq_sb = constp.tile([128, CH, HDIM], bf16, tag="wq", name="wq")
            nc.sync.dma_start(wq_sb[:], wq_r)
            wk_sb = constp.tile([128, CH, HDIM], bf16, tag="wk", name="wk")
            nc.sync.dma_start(wk_sb[:], wk_r)
            wv_sb = constp.tile([128, CH, HDIM], bf16, tag="wv", name="wv")
            nc.sync.dma_start(wv_sb[:], wv_r)
            bqk_sb = constp.tile([128, 2], f32, tag="bqk", name="bqk")
            nc.sync.dma_start(bqk_sb[:], bias_qk.ap())
            wo_sb = constp.tile([HDIM, D], bf16, tag="wo", name="wo")
            masks_sb = constp.tile([128, 4, SQ], bf16, tag="masks", name="masks")
            bv_sb = constp.tile([128, 2, HD], f32, tag="bv", name="bv")

            # ---- PE p-state warmup: tiny matmuls on the ones row while the
            # first x tiles are still in flight on DMA ----
            warm = ps_m.tile([128, SQ], f32, tag="m", name="m")
            for _ in range(16):
                nc.tensor.matmul(warm[0:64, 0:64], ones_sb[:], ones_sb[:],
                                 start=True, stop=True)

            # ---- filler machinery ----
            # fillq: generators yielding after each matmul (PE-side steps).
            # epiq: deferred ACT/DVE epilogues (PSUM->SBUF copies); draining
            # them only at sub-block boundaries keeps the in-order ScalarE
            # queue clean for the exp chain. fill() pops one epilogue early
            # when >=2 are pending so ps_m slots keep rotating.
            fillq = deque()
            epiq = deque()

            def fill(n):
                k = 0
                if not FILLERS:
                    n = None
                while fillq and (n is None or k < n):
                    if len(epiq) >= 2:
                        epiq.popleft()()
                    try:
                        next(fillq[0])
                        k += 1
                    except StopIteration:
                        fillq.popleft()
                if n is None:
                    while epiq:
                        epiq.popleft()()

            def fill_epi():
                while epiq:
                    epiq.popleft()()

            def qk_group(t, so, which):
                sl = slice(so * SQ, (so + 1) * SQ)
                w = wq_sb if which == 0 else wk_sb
                dst = t["qt"] if which == 0 else t["kt"]
                ps = ps_m.tile([128, SQ], f32, tag="m", name="m")
                for c in range(CH):
                    nc.tensor.matmul(ps[:], w[:, c, :], t["xt"][so][:, c, :],
                                     start=(c == 0), stop=(c == CH - 1))
                    if c < CH - 1:
                        yield
                if with_qkbias:
                    epiq.append(lambda: nc.scalar.add(dst[:, sl], ps[:],
                                                      bqk_sb[:, which:which + 1]))
                else:
                    epiq.append(lambda: nc.scalar.copy(dst[:, sl], ps[:]))
                yield

            def v_group(t, st):
                so, off = divmod(st * SK, SQ)
                ps = ps_m.tile([128, SQ], f32, tag="m", name="m")
                for c in range(CH):
                    nc.tensor.matmul(ps[:, 0:HDIM],
                                     t["xt"][so][:, c, off:off + SK],
                                     wv_sb[:, c, :],
                                     start=(c == 0), stop=(c == CH - 1))
                    if c < CH - 1:
                        yield

                def epi():
                    nc.vector.tensor_copy(
                        t["vb"][:, st, :, 0:HD],
                        ps[:, 0:HDIM].rearrange("p (h d) -> p h d", d=HD))
                    if with_vbias:
                        nc.vector.tensor_add(t["vb"][:, st, :, 0:HD],
                                             t["vb"][:, st, :, 0:HD], bv_sb[:])
                epiq.append(epi)
                yield

            def proj_group(t, b, m, so):
                sl = slice(so * SQ, (so + 1) * SQ)
                # the final q-block keeps per-chunk DMAs (shorter tail);
                # all others batch the 8 chunk copies into one DMA issue to
                # keep the Sync engine's descriptor queue short.
                solo = b == B - 1 and so == NQ - 1
                # the tail clump runs after the last attention pair, so the
                # score banks are free: borrow them for odd chunks to double
                # the PSUM slots the drain rotates through
                if solo and m % 2 == 1:
                    ps = ps_s.tile([128, 2, SQ], f32, tag="s", name="s")[:, 0, :]
                else:
                    ps = ps_m.tile([128, SQ], f32, tag="m", name="m")
                nc.tensor.matmul(ps, wo_sb[:, m * 128:(m + 1) * 128],
                                 t["at"][:, sl], start=True, stop=True)

                def epi():
                    if m == 0:
                        t[("y", so)] = yp.tile([128, CH, SQ], bf16,
                                               tag="y", name="y")
                    y_sb = t[("y", so)]
                    dst = y_sb[:, m, :]
                    act_mod = 2 if solo else 4
                    if m % act_mod == 0:
                        nc.scalar.copy(dst, ps)
                    else:
                        nc.vector.tensor_copy(dst, ps)
                    # single batched DMA also for the tail q-block: the old
                    # per-chunk DMAs each stalled the Sync queue ~10us+
                    # behind their chunk-copy semaphores, serializing the
                    # drain at ~2.5us/chunk with the PE gone cold
                    if m == CH - 1:
                        nc.sync.dma_start(out_r[b, :, :, sl], y_sb[:])
                epiq.append(epi)
                yield

            tiles = {}

            def start_batch(b):
                xt_cs = [None] * NQ
                for so in range(NQ):
                    xc = xtp.tile([128, CH, SQ], bf16, tag=f"xt{so}",
                                  name=f"xt{so}")
                    sl = slice(so * SQ, (so + 1) * SQ)
                    # one batched DMA per q-block for EVERY batch: batch 0's
                    # old per-chunk DMAs were meant to cut latency, but the
                    # trace shows the 32 descriptor issues trickle out to
                    # ~36us on the Sync queue (each carries a ~10us wait),
                    # so the startup chains were x-ISSUE-paced; 4 batched
                    # issues land all of x by ~16us instead
                    nc.sync.dma_start(xc[:], xt_r[b, :, :, sl])
                    xt_cs[so] = xc
                t = {
                    "xt": xt_cs,
                    "qt": qkp.tile([128, S], bf16, tag="qt", name="qt"),
                    "kt": qkp.tile([128, S], bf16, tag="kt", name="kt"),
                    "vb": qkp.tile([128, NKT, 2, 65], bf16, tag="vb", name="vb"),
                }
                tiles[b] = t
                nc.vector.memset(t["vb"][:, :, :, HD:65], 1.0)
                # interleave the short v chains between the long qk chains so
                # the v groups' ps_m slot turnarounds hide under qk streaming
                for so in range(NQ):
                    fillq.append(qk_group(t, so, 0))
                    fillq.append(v_group(t, 4 * so + 0))
                    fillq.append(v_group(t, 4 * so + 1))
                    fillq.append(qk_group(t, so, 1))
                    fillq.append(v_group(t, 4 * so + 2))
                    fillq.append(v_group(t, 4 * so + 3))

            def attention(b):
                t = tiles[b]
                t["at"] = qkp.tile([128, S], bf16, tag="at", name="at")
                at = t["at"]
                qt, kt, vb = t["qt"], t["kt"], t["vb"]
                # non-final batches DESCENDING (16-k-tile block sees the
                # fullest filler queue); the final batch has no next-batch
                # chains, so it runs ASCENDING: each q-block's out-projs
                # interleave into the next (bigger) q-block's attention and
                # only the very last q-block's projs drain solo
                order = range(NQ) if b == B - 1 else range(NQ - 1, -1, -1)
                for qi in order:
                    qsl = slice(qi * SQ, (qi + 1) * SQ)
                    for h in range(HPC):
                        hsl = slice(h * HD, (h + 1) * HD)
                        n_kt = qi * 4 + 4
                        n_pairs = n_kt // 2
                        pso = ps_o.tile([65, SQ], f32, tag="o", name="o")
                        prevs = deque()

                        def emit_pv(e0, p0, c0s):
                            for j in range(2):
                                ki = 2 * p0 + j
                                nc.tensor.matmul(pso[:, c0s[j]:SQ], vb[:, ki, h, :],
                                                 e0[:, j, c0s[j]:SQ],
                                                 start=(ki == 0),
                                                 stop=(ki == n_kt - 1))

                        for pi in range(n_pairs):
                            # diagonal k-tile at delta d: queries < d in this
                            # block are fully masked -> skip those columns.
                            # The pair's exp covers from the smaller c0; the
                            # skipped-but-exp'd region holds stale bounded
                            # scores and is zeroed by the mask multiply.
                            c0s = []
                            for j in range(2):
                                didx = 2 * pi + j - qi * 4
                                c0s.append(didx * SK if didx > 0 else 0)
                            ce = min(c0s)
                            psp = ps_s.tile([128, 2, SQ], f32, tag="s", name="s")
                            for j in range(2):
                                ki = 2 * pi + j
                                nc.tensor.matmul(psp[:, j, c0s[j]:SQ],
                                                 kt[hsl, ki * SK:(ki + 1) * SK],
                                                 qt[hsl, qi * SQ + c0s[j]:(qi + 1) * SQ],
                                                 start=True, stop=True)
                            fill(1)
                            epair = ep.tile([128, 2, SQ], bf16, tag="e", name="e")
                            nc.scalar.activation(epair[:, :, ce:SQ], psp[:, :, ce:SQ], EXP)
                            for j in range(2):
                                didx = 2 * pi + j - qi * 4
                                if didx >= 0:
                                    dd = didx * SK
                                    nc.vector.tensor_mul(
                                        epair[:, j, dd:dd + SK],
                                        epair[:, j, dd:dd + SK],
                                        masks_sb[:, didx, dd:dd + SK])
                            fill(1)
                            if len(prevs) >= PV_LAG:
                                emit_pv(*prevs.popleft())
                                fill(1)
                            prevs.append((epair, pi, c0s))
                        while prevs:
                            emit_pv(*prevs.popleft())

                        # normalize: at[hd, q] = num[hd, q] * bcast(1/den[q])
                        recip = smallp.tile([1, SQ], f32, tag="recip", name="recip")
                        if FAST_RECIP:
                            den = smallp.tile([1, SQ], f32, tag="den", name="den")
                            nc.vector.tensor_copy(den[:], pso[64:65, :])
                            nc.vector.reciprocal_approx_fast(out=recip[:],
                                                             in_=den[:])
                        else:
                            nc.vector.reciprocal(recip[:], pso[64:65, :])
                        bc = smallp.tile([64, SQ], f32, tag="bc", name="bc")
                        nc.gpsimd.partition_broadcast(bc[:], recip[:], channels=64)
                        nc.vector.tensor_mul(at[hsl, qsl], pso[0:64, :], bc[:])
                        fill_epi()
                        fill(4)
                    for m in range(CH):
                        fillq.append(proj_group(t, b, m, qi))
                    fill_epi()
                fill(None)

            start_batch(0)
            nc.sync.dma_start(masks_sb[:], masks.ap().rearrange("p (d q) -> p d q", q=SQ))
            nc.sync.dma_start(wo_sb[:], wo.ap())
            nc.sync.dma_start(bv_sb[:], bias_v.ap().rearrange("p (h d) -> p h d", d=HD))
            fill(None)
            for b in range(B):
                if b + 1 < B:
                    start_batch(b + 1)
                attention(b)

    nc.compile()
    return nc


def _get_nc(with_vbias=False, with_qkbias=False):
    key = ("nc", with_vbias, with_qkbias, FAST_RECIP, FILLERS)
    if key not in _CACHE:
        _CACHE[key] = _build(with_vbias, with_qkbias)
    return _CACHE[key]


def _prep_in_maps(x, w_in, b_in, w_out):
    bf16 = ml_dtypes.bfloat16
    scale = 1.0 / np.sqrt(HD)
    xt_host = np.ascontiguousarray(x.transpose(0, 2, 1)).astype(bf16)

    # mask[p, d*SQ + q] = 1 if key (d*128 + p) <= query q within the block
    p_idx = np.arange(128)[:, None]
    q_idx = np.arange(SQ)[None, :]
    mask_host = np.concatenate(
        [(p_idx + d * SK <= q_idx) for d in range(4)], axis=1).astype(bf16)
    ones_host = np.ones((1, 64), np.float32)

    in_maps = []
    for c in range(N_CORES):
        cs = c * HDIM
        wq_c = np.ascontiguousarray(w_in[:, cs:cs + HDIM] * scale).astype(bf16)
        wk_c = np.ascontiguousarray(w_in[:, D + cs:D + cs + HDIM]).astype(bf16)
        wv_c = np.ascontiguousarray(w_in[:, 2 * D + cs:2 * D + cs + HDIM]).astype(bf16)
        wo_c = np.ascontiguousarray(w_out[cs:cs + HDIM, :]).astype(bf16)
        bqk_c = np.ascontiguousarray(
            np.stack([b_in[cs:cs + HDIM] * scale,
                      b_in[D + cs:D + cs + HDIM]], axis=1).astype(np.float32))
        bv_c = np.ascontiguousarray(
            np.broadcast_to(b_in[2 * D + cs:2 * D + cs + HDIM],
                            (128, HDIM)).astype(np.float32))
        in_maps.append({
            "xt": xt_host, "wq": wq_c, "wk": wk_c, "wv": wv_c, "wo": wo_c,
            "masks": mask_host, "bias_qk": bqk_c, "bias_v": bv_c,
            "ones64": ones_host,
        })
    return in_maps


def kernel(x, w_in, b_in, w_out, b_out):
    from concourse.bass_utils import run_bass_kernel_spmd

    x = np.asarray(x, dtype=np.float32)
    w_in = np.asarray(w_in, dtype=np.float32)
    b_in = np.asarray(b_in, dtype=np.float32)
    w_out = np.asarray(w_out, dtype=np.float32)
    b_out = np.asarray(b_out, dtype=np.float32)

    with_vbias = bool(np.any(b_in[2 * D:]))
    with_qkbias = bool(np.any(b_in[:2 * D]))
    nc = _get_nc(with_vbias, with_qkbias)
    in_maps = _prep_in_maps(x, w_in, b_in, w_out)
    _CACHE["in_maps"] = in_maps

    res = run_bass_kernel_spmd(nc, in_maps, core_ids=list(range(N_CORES)))
    y_t = res.results[0]["out"].astype(np.float32)
    for c in range(1, N_CORES):
        y_t += res.results[c]["out"].astype(np.float32)
    y = y_t.transpose(0, 2, 1).astype(np.float32) + b_out
    return y



# revision 46
# speedup vs baseline: 1.0500x; 1.0099x over previous
"""Multi-head causal attention block on 8 trn2 NeuronCores.

Sharding: tensor-parallel over heads (16 heads / 8 cores = 2 heads per core).
Each core gets the full x (pre-transposed on host), its 128-wide slice of the
QKV projection columns and of the w_out rows, computes its 2 heads end to end,
and emits a partial y^T = (attn_out @ w_out_slice)^T in bf16.  Host sums the
8 partials (the "all-reduce"), transposes back, adds b_out.

Device layout (everything "transposed": head-dim on partitions, seq free):
  x^T    [128p, 8, 2048]   Q^T,K^T [128p, 2048]   V [128p(s), 16, 2, 65]
  (V natural, per head 64 hd cols + ones column so the PV matmul accumulates
  the softmax denominator for free).  scores^T [128 keys, 512 q] in PSUM; exp
  on ScalarE without max subtraction (scores ~ N(0,1)); static causal {0,1}
  masks multiplied in on VectorE for diagonal k-tiles; fully-masked tiles
  skipped.  attn_out^T = numerator^T * bcast(1/den): the denominator row is
  staged to SBUF (the custom-DVE fast reciprocal reads PSUM@partition-64
  wrong), reciprocal'd with reciprocal_approx_fast, and partition-broadcast
  on GpSimd.  Diagonal k-tiles skip fully-masked query columns (128-col
  granularity); columns below the band are never read.

Engine budget per batch: PE 56us | ACT 41us of exp + epilogues-in-holes |
DVE ~40us | Pool ~9us.  The ScalarE exp chain is gated by its own QK pairs,
so it naturally idles between exps; qk-bias epilogues and 1-in-4 projection
copies (1-in-2 on the last batch) slot into those holes, the rest of the
PSUM->SBUF copies (V tiles, most projections, den staging) go to VectorE.
Mask multiplies stay on VectorE: GpSimd's ~0.8us wake latency on the
exp->mask->PV edge collapses the PE p-state (measured 2.2x slowdown).
PV lags its exp by THREE pairs so one stray ScalarE epilogue cannot stall
the in-order PE queue.

Scheduling: the attention pair-loop (QK pair -> exp pair -> PV pair) stalls
TensorE while ScalarE exps.  Independent matmuls -- the NEXT batch's QKV
projections and the finished q-blocks' output projections -- are kept in a
FIFO of generators and dripped into those gaps, keeping TensorE dense and the
PE clock at 2.4 GHz.  The short V-projection chains interleave between the
long QK-projection chains so their ps_m slot turnarounds hide under qk
streaming.  A warmup burst of 1x64 matmuls on the ones row ramps the PE
p-state while the first x tiles are still in flight on DMA.  Q-blocks run in
DESCENDING size order for non-final batches (the 16-k-tile blocks -- most
filler slots -- run while the queue is fullest); the FINAL batch runs
ASCENDING, since it has no next-batch chains to fill with: each q-block's
out-projections interleave into the next (bigger) q-block's attention and
only the very last q-block's projs drain solo (borrowing the freed score
banks for PSUM rotation).

DMA: the Sync engine spends ~0.7us per issued descriptor, so transfers are
batched -- x arrives as one [128, 8, 512] tile per query block (per-chunk
for batch 0, which is latency-critical), and the projection outputs of a
query block are collected into one [128, 8, 512] bf16 tile and shipped with
a single DMA.  (Per-chunk tail DMAs were tried and are ~35us SLOWER: each
waits ~10us on its chunk-copy semaphore in the Sync queue, serializing the
drain at ~2.5us/chunk with the PE dropped to its cold p-state.)
Partials are emitted in bf16 (the host all-reduce absorbs the rounding).
"""

from collections import deque

import numpy as np
import ml_dtypes

B, S, D, H = 4, 2048, 1024, 16
HD = 64                      # head dim
N_CORES = 8
HPC = H // N_CORES           # heads per core = 2
HDIM = HPC * HD              # per-core qkv slice width = 128
CH = D // 128                # contraction chunks = 8
SQ = 512                     # query block
NQ = S // SQ                 # 4 query blocks
SK = 128                     # key tile
NKT = S // SK                # 16 key tiles

_CACHE = {}
FAST_RECIP = True
FILLERS = True
PV_LAG = 3


def _build(with_vbias, with_qkbias=False):
    import concourse.bass as bass
    import concourse.tile as tile
    from concourse import bacc, mybir
    from contextlib import ExitStack

    bf16 = mybir.dt.bfloat16
    f32 = mybir.dt.float32
    EXP = mybir.ActivationFunctionType.Exp

    nc = bacc.Bacc("TRN2", target_bir_lowering=False, debug=False,
                   num_devices=N_CORES)

    xt = nc.dram_tensor("xt", [B, D, S], bf16, kind="ExternalInput")
    wq = nc.dram_tensor("wq", [D, HDIM], bf16, kind="ExternalInput")
    wk = nc.dram_tensor("wk", [D, HDIM], bf16, kind="ExternalInput")
    wv = nc.dram_tensor("wv", [D, HDIM], bf16, kind="ExternalInput")
    wo = nc.dram_tensor("wo", [HDIM, D], bf16, kind="ExternalInput")
    masks = nc.dram_tensor("masks", [128, 4 * SQ], bf16, kind="ExternalInput")
    bias_qk = nc.dram_tensor("bias_qk", [128, 2], f32, kind="ExternalInput")
    bias_v = nc.dram_tensor("bias_v", [128, 2 * HD], f32, kind="ExternalInput")
    ones64 = nc.dram_tensor("ones64", [1, 64], f32, kind="ExternalInput")
    out = nc.dram_tensor("out", [B, D, S], bf16, kind="ExternalOutput")

    xt_r = xt.ap().rearrange("b (o p) s -> b p o s", p=128)
    wq_r = wq.ap().rearrange("(o p) m -> p o m", p=128)
    wk_r = wk.ap().rearrange("(o p) m -> p o m", p=128)
    wv_r = wv.ap().rearrange("(o p) m -> p o m", p=128)
    out_r = out.ap().rearrange("b (o p) s -> b p o s", p=128)

    with tile.TileContext(nc) as tc:
        with ExitStack() as ctx:
            constp = ctx.enter_context(tc.tile_pool(name="const", bufs=1))
            xtp = ctx.enter_context(tc.tile_pool(name="xt", bufs=2))
            qkp = ctx.enter_context(tc.tile_pool(name="qk", bufs=2))
            ep = ctx.enter_context(tc.tile_pool(name="e", bufs=8))
            smallp = ctx.enter_context(tc.tile_pool(name="small", bufs=3))
            yp = ctx.enter_context(tc.tile_pool(name="y", bufs=3))
            ps_s = ctx.enter_context(tc.tile_pool(name="ps_s", bufs=2, space="PSUM"))
            ps_o = ctx.enter_context(tc.tile_pool(name="ps_o", bufs=2, space="PSUM"))
            ps_m = ctx.enter_context(tc.tile_pool(name="ps_m", bufs=2, space="PSUM"))

            # ---- constants (ones first: it feeds the PE warmup loop) ----
            ones_sb = constp.tile([1, 64], f32, tag="ones", name="ones")
            nc.sync.dma_start(ones_sb[:], ones64.ap())
            wq_sb = constp.tile([128, CH, HDIM], bf16, tag="wq", name="wq")
            nc.sync.dma_start(wq_sb[:], wq_r)
            wk_sb = constp.tile([128, CH, HDIM], bf16, tag="wk", name="wk")
            nc.sync.dma_start(wk_sb[:], wk_r)
            wv_sb = constp.tile([128, CH, HDIM], bf16, tag="wv", name="wv")
            nc.sync.dma_start(wv_sb[:], wv_r)
            bqk_sb = constp.tile([128, 2], f32, tag="bqk", name="bqk")
            nc.sync.dma_start(bqk_sb[:], bias_qk.ap())
            wo_sb = constp.tile([HDIM, D], bf16, tag="wo", name="wo")
            masks_sb = constp.tile([128, 4, SQ], bf16, tag="masks", name="masks")
            bv_sb = constp.tile([128, 2, HD], f32, tag="bv", name="bv")

            # ---- PE p-state warmup: tiny matmuls on the ones row while the
            # first x tiles are still in flight on DMA ----
            warm = ps_m.tile([128, SQ], f32, tag="m", name="m")
            for _ in range(16):
                nc.tensor.matmul(warm[0:64, 0:64], ones_sb[:], ones_sb[:],
                                 start=True, stop=True)

            # ---- filler machinery ----
            # fillq: generators yielding after each matmul (PE-side steps).
            # epiq: deferred ACT/DVE epilogues (PSUM->SBUF copies); draining
            # them only at sub-block boundaries keeps the in-order ScalarE
            # queue clean for the exp chain. fill() pops one epilogue early
            # when >=2 are pending so ps_m slots keep rotating.
            fillq = deque()
            epiq = deque()

            def fill(n):
                k = 0
                if not FILLERS:
                    n = None
                while fillq and (n is None or k < n):
                    if len(epiq) >= 2:
                        epiq.popleft()()
                    try:
                        next(fillq[0])
                        k += 1
                    except StopIteration:
                        fillq.popleft()
                if n is None:
                    while epiq:
                        epiq.popleft()()

            def fill_epi():
                while epiq:
                    epiq.popleft()()

            def qk_group(t, so, which):
                sl = slice(so * SQ, (so + 1) * SQ)
                w = wq_sb if which == 0 else wk_sb
                dst = t["qt"] if which == 0 else t["kt"]
                ps = ps_m.tile([128, SQ], f32, tag="m", name="m")
                for c in range(CH):
                    nc.tensor.matmul(ps[:], w[:, c, :], t["xt"][so][:, c, :],
                                     start=(c == 0), stop=(c == CH - 1))
                    if c < CH - 1:
                        yield
                if with_qkbias:
                    epiq.append(lambda: nc.scalar.add(dst[:, sl], ps[:],
                                                      bqk_sb[:, which:which + 1]))
                else:
                    epiq.append(lambda: nc.scalar.copy(dst[:, sl], ps[:]))
                yield

            def v_group(t, st):
                so, off = divmod(st * SK, SQ)
                ps = ps_m.tile([128, SQ], f32, tag="m", name="m")
                for c in range(CH):
                    nc.tensor.matmul(ps[:, 0:HDIM],
                                     t["xt"][so][:, c, off:off + SK],
                                     wv_sb[:, c, :],
                                     start=(c == 0), stop=(c == CH - 1))
                    if c < CH - 1:
                        yield

                def epi():
                    nc.vector.tensor_copy(
                        t["vb"][:, st, :, 0:HD],
                        ps[:, 0:HDIM].rearrange("p (h d) -> p h d", d=HD))
                    if with_vbias:
                        nc.vector.tensor_add(t["vb"][:, st, :, 0:HD],
                                             t["vb"][:, st, :, 0:HD], bv_sb[:])
                epiq.append(epi)
                yield

            def proj_group(t, b, m, so):
                sl = slice(so * SQ, (so + 1) * SQ)
                # the final q-block keeps per-chunk DMAs (shorter tail);
                # all others batch the 8 chunk copies into one DMA issue to
                # keep the Sync engine's descriptor queue short.
                solo = b == B - 1 and so == NQ - 1
                # the tail clump runs after the last attention pair, so the
                # score banks are free: borrow them for odd chunks to double
                # the PSUM slots the drain rotates through
                if solo and m % 2 == 1:
                    ps = ps_s.tile([128, 2, SQ], f32, tag="s", name="s")[:, 0, :]
                else:
                    ps = ps_m.tile([128, SQ], f32, tag="m", name="m")
                nc.tensor.matmul(ps, wo_sb[:, m * 128:(m + 1) * 128],
                                 t["at"][:, sl], start=True, stop=True)

                def epi():
                    if m == 0:
                        t[("y", so)] = yp.tile([128, CH, SQ], bf16,
                                               tag="y", name="y")
                    y_sb = t[("y", so)]
                    dst = y_sb[:, m, :]
                    act_mod = 2 if solo else 4
                    if m % act_mod == 0:
                        nc.scalar.copy(dst, ps)
                    else:
                        nc.vector.tensor_copy(dst, ps)
                    # single batched DMA also for the tail q-block: the old
                    # per-chunk DMAs each stalled the Sync queue ~10us+
                    # behind their chunk-copy semaphores, serializing the
                    # drain at ~2.5us/chunk with the PE gone cold
                    if m == CH - 1:
                        nc.sync.dma_start(out_r[b, :, :, sl], y_sb[:])
                epiq.append(epi)
                yield

            tiles = {}

            def start_batch(b):
                xt_cs = [None] * NQ
                for so in range(NQ):
                    xc = xtp.tile([128, CH, SQ], bf16, tag=f"xt{so}",
                                  name=f"xt{so}")
                    sl = slice(so * SQ, (so + 1) * SQ)
                    if b == 0:
                        # batch 0 is latency-critical: per-chunk DMAs so the
                        # first qk chains start as chunks arrive
                        for c in range(CH):
                            nc.sync.dma_start(xc[:, c, :], xt_r[b, :, c, sl])
                    else:
                        nc.sync.dma_start(xc[:], xt_r[b, :, :, sl])
                    xt_cs[so] = xc
                t = {
                    "xt": xt_cs,
                    "qt": qkp.tile([128, S], bf16, tag="qt", name="qt"),
                    "kt": qkp.tile([128, S], bf16, tag="kt", name="kt"),
                    "vb": qkp.tile([128, NKT, 2, 65], bf16, tag="vb", name="vb"),
                }
                tiles[b] = t
                nc.vector.memset(t["vb"][:, :, :, HD:65], 1.0)
                # interleave the short v chains between the long qk chains so
                # the v groups' ps_m slot turnarounds hide under qk streaming
                for so in range(NQ):
                    fillq.append(qk_group(t, so, 0))
                    fillq.append(v_group(t, 4 * so + 0))
                    fillq.append(v_group(t, 4 * so + 1))
                    fillq.append(qk_group(t, so, 1))
                    fillq.append(v_group(t, 4 * so + 2))
                    fillq.append(v_group(t, 4 * so + 3))

            def attention(b):
                t = tiles[b]
                t["at"] = qkp.tile([128, S], bf16, tag="at", name="at")
                at = t["at"]
                qt, kt, vb = t["qt"], t["kt"], t["vb"]
                # non-final batches DESCENDING (16-k-tile block sees the
                # fullest filler queue); the final batch has no next-batch
                # chains, so it runs ASCENDING: each q-block's out-projs
                # interleave into the next (bigger) q-block's attention and
                # only the very last q-block's projs drain solo
                order = range(NQ) if b == B - 1 else range(NQ - 1, -1, -1)
                for qi in order:
                    qsl = slice(qi * SQ, (qi + 1) * SQ)
                    for h in range(HPC):
                        hsl = slice(h * HD, (h + 1) * HD)
                        n_kt = qi * 4 + 4
                        n_pairs = n_kt // 2
                        pso = ps_o.tile([65, SQ], f32, tag="o", name="o")
                        prevs = deque()

                        def emit_pv(e0, p0, c0s):
                            for j in range(2):
                                ki = 2 * p0 + j
                                nc.tensor.matmul(pso[:, c0s[j]:SQ], vb[:, ki, h, :],
                                                 e0[:, j, c0s[j]:SQ],
                                                 start=(ki == 0),
                                                 stop=(ki == n_kt - 1))

                        for pi in range(n_pairs):
                            # diagonal k-tile at delta d: queries < d in this
                            # block are fully masked -> skip those columns.
                            # The pair's exp covers from the smaller c0; the
                            # skipped-but-exp'd region holds stale bounded
                            # scores and is zeroed by the mask multiply.
                            c0s = []
                            for j in range(2):
                                didx = 2 * pi + j - qi * 4
                                c0s.append(didx * SK if didx > 0 else 0)
                            ce = min(c0s)
                            psp = ps_s.tile([128, 2, SQ], f32, tag="s", name="s")
                            for j in range(2):
                                ki = 2 * pi + j
                                nc.tensor.matmul(psp[:, j, c0s[j]:SQ],
                                                 kt[hsl, ki * SK:(ki + 1) * SK],
                                                 qt[hsl, qi * SQ + c0s[j]:(qi + 1) * SQ],
                                                 start=True, stop=True)
                            fill(1)
                            epair = ep.tile([128, 2, SQ], bf16, tag="e", name="e")
                            nc.scalar.activation(epair[:, :, ce:SQ], psp[:, :, ce:SQ], EXP)
                            for j in range(2):
                                didx = 2 * pi + j - qi * 4
                                if didx >= 0:
                                    dd = didx * SK
                                    nc.vector.tensor_mul(
                                        epair[:, j, dd:dd + SK],
                                        epair[:, j, dd:dd + SK],
                                        masks_sb[:, didx, dd:dd + SK])
                            fill(1)
                            if len(prevs) >= PV_LAG:
                                emit_pv(*prevs.popleft())
                                fill(1)
                            prevs.append((epair, pi, c0s))
                        while prevs:
                            emit_pv(*prevs.popleft())

                        # normalize: at[hd, q] = num[hd, q] * bcast(1/den[q])
                        recip = smallp.tile([1, SQ], f32, tag="recip", name="recip")
                        if FAST_RECIP:
                            den = smallp.tile([1, SQ], f32, tag="den", name="den")
                            nc.vector.tensor_copy(den[:], pso[64:65, :])
                            nc.vector.reciprocal_approx_fast(out=recip[:],
                                                             in_=den[:])
                        else:
                            nc.vector.reciprocal(recip[:], pso[64:65, :])
                        bc = smallp.tile([64, SQ], f32, tag="bc", name="bc")
                        nc.gpsimd.partition_broadcast(bc[:], recip[:], channels=64)
                        nc.vector.tensor_mul(at[hsl, qsl], pso[0:64, :], bc[:])
                        fill_epi()
                        fill(4)
                    for m in range(CH):
                        fillq.append(proj_group(t, b, m, qi))
                    fill_epi()
                fill(None)

            start_batch(0)
            nc.sync.dma_start(masks_sb[:], masks.ap().rearrange("p (d q) -> p d q", q=SQ))
            nc.sync.dma_start(wo_sb[:], wo.ap())
            nc.sync.dma_start(bv_sb[:], bias_v.ap().rearrange("p (h d) -> p h d", d=HD))
            fill(None)
            for b in range(B):
                if b + 1 < B:
                    start_batch(b + 1)
                attention(b)

    nc.compile()
    return nc


def _get_nc(with_vbias=False, with_qkbias=False):
    key = ("nc", with_vbias, with_qkbias, FAST_RECIP, FILLERS)
    if key not in _CACHE:
        _CACHE[key] = _build(with_vbias, with_qkbias)
    return _CACHE[key]


def _prep_in_maps(x, w_in, b_in, w_out):
    bf16 = ml_dtypes.bfloat16
    scale = 1.0 / np.sqrt(HD)
    xt_host = np.ascontiguousarray(x.transpose(0, 2, 1)).astype(bf16)

    # mask[p, d*SQ + q] = 1 if key (d*128 + p) <= query q within the block
    p_idx = np.arange(128)[:, None]
    q_idx = np.arange(SQ)[None, :]
    mask_host = np.concatenate(
        [(p_idx + d * SK <= q_idx) for d in range(4)], axis=1).astype(bf16)
    ones_host = np.ones((1, 64), np.float32)

    in_maps = []
    for c in range(N_CORES):
        cs = c * HDIM
        wq_c = np.ascontiguousarray(w_in[:, cs:cs + HDIM] * scale).astype(bf16)
        wk_c = np.ascontiguousarray(w_in[:, D + cs:D + cs + HDIM]).astype(bf16)
        wv_c = np.ascontiguousarray(w_in[:, 2 * D + cs:2 * D + cs + HDIM]).astype(bf16)
        wo_c = np.ascontiguousarray(w_out[cs:cs + HDIM, :]).astype(bf16)
        bqk_c = np.ascontiguousarray(
            np.stack([b_in[cs:cs + HDIM] * scale,
                      b_in[D + cs:D + cs + HDIM]], axis=1).astype(np.float32))
        bv_c = np.ascontiguousarray(
            np.broadcast_to(b_in[2 * D + cs:2 * D + cs + HDIM],
                            (128, HDIM)).astype(np.float32))
        in_maps.append({
            "xt": xt_host, "wq": wq_c, "wk": wk_c, "wv": wv_c, "wo": wo_c,
            "masks": mask_host, "bias_qk": bqk_c, "bias_v": bv_c,
            "ones64": ones_host,
        })
    return in_maps


def kernel(x, w_in, b_in, w_out, b_out):
    from concourse.bass_utils import run_bass_kernel_spmd

    x = np.asarray(x, dtype=np.float32)
    w_in = np.asarray(w_in, dtype=np.float32)
    b_in = np.asarray(b_in, dtype=np.float32)
    w_out = np.asarray(w_out, dtype=np.float32)
    b_out = np.asarray(b_out, dtype=np.float32)

    with_vbias = bool(np.any(b_in[2 * D:]))
    with_qkbias = bool(np.any(b_in[:2 * D]))
    nc = _get_nc(with_vbias, with_qkbias)
    in_maps = _prep_in_maps(x, w_in, b_in, w_out)
    _CACHE["in_maps"] = in_maps

    res = run_bass_kernel_spmd(nc, in_maps, core_ids=list(range(N_CORES)))
    y_t = res.results[0]["out"].astype(np.float32)
    for c in range(1, N_CORES):
        y_t += res.results[c]["out"].astype(np.float32)
    y = y_t.transpose(0, 2, 1).astype(np.float32) + b_out
    return y

